# revision 35
# baseline (speedup 1.0000x reference)
"""Trainium2 Bass kernel for the ButterflyMlp problem.

Computes log_softmax(L3(relu(L2(relu(L1(x)))))) where each Li is a masked
linear layer (butterfly sparsity: global column stripes + a diagonal band),
batch 65536, data-parallel over 8 NeuronCores (8192 rows/core).

Strategy (per core, feature-major throughout):
  - Masks are pre-applied to weights on host. Layer-1 exploits the butterfly
    structure: the stripe columns (mask true for every output row) form a
    dense [|S|, 784] GEMM shared by all outputs, and the per-output-block
    band adds one narrow [|R_j|<=128, 112] GEMM per 112-row output block.
  - Layers 1+2 run in fp8 e4m3 with DoubleRow perf mode: the PE array
    virtualizes to 256 contraction rows, so the K=204 stripe GEMM and the
    K=224 layer-2 block pairs each take a single matmul pass. Layer 3 and
    the log-softmax epilogue stay fp16 (measured end-to-end max rel err
    ~1.1e-2 vs the 2e-2 gate).
  - Bulk data moves through SWDGE (nc.gpsimd.dma_start), striped across the
    16 SDMA engines; DMAs are emitted in consumption order and the first
    band tile is split so compute starts as soon as the first slices land.
  - PE warmup matmuls on a scratch tile run during the framework preamble +
    first DMA transfers so the HAM clock gate reaches 8/8 before real work.
  - ReLU+bias fuses into the PSUM->SBUF eviction, alternating ScalarE and
    VectorE; layer-1 blocks evict into one [112, 7, 512] fp8 tile that
    layer-2 consumes in DoubleRow pairs.
  - log_softmax stays feature-major: exp (ACT) -> all-ones matmul (sums the
    10 class partitions and broadcasts) -> ln (ACT) -> subtract (DVE).
  - Output is [10, 8192] per core; host transposes and concatenates.
"""
import sys
sys.path.insert(0, "/opt/trn_rl_repo")
import numpy as np
import ml_dtypes

import concourse.bass as bass
import concourse.bacc as bacc
import concourse.mybir as mybir
import concourse.tile as tile
import concourse.bass_isa as bass_isa
from concourse import bass_utils

import os
F32 = mybir.dt.float32
F32R = mybir.dt.float16           # non-fp8 matmul operand dtype
F8 = mybir.dt.float8e4
FP8 = os.environ.get("BUTTERFLY_FP8", "1") == "1"
AF = mybir.ActivationFunctionType
ALU = mybir.AluOpType
DR = mybir.MatmulPerfMode.DoubleRow

# All activation functions this kernel uses live together in the
# natural_log_exp_and_others table set, but the greedy per-function set
# chooser picks exp_and_others for Exp and natural_log* for Ln, reloading
# ACT tables twice per chunk (~1.3us each). Restrict every other set's
# advertised contents so the chooser lands on the one set that covers
# everything and emits a single load. Set ids stay valid: the dict keys
# and order are unchanged.
_PIN_SET = "natural_log_exp_and_others"
_orig_gat = bacc.get_activation_tables


def _pinned_gat(arch):
    tabs = _orig_gat(arch)
    need = {AF.Relu, AF.Identity, AF.Exp, AF.Ln, AF.Copy}
    if _PIN_SET in tabs and need <= tabs[_PIN_SET]:
        for name in tabs:
            if name != _PIN_SET:
                tabs[name] = tabs[name] - need
    return tabs


bacc.get_activation_tables = _pinned_gat

N_CORES = 8
NB = 512          # batch columns per matmul (one PSUM bank of fp32)
SC = 512          # batch columns per DMA superchunk / epilogue batch
OT = 112          # layer-1 output block width (784/7; band window fits 128)
N_WARM = 8        # PE warmup matmuls issued while the first DMAs land


def _decompose_mask1(mask1):
    """Split the butterfly mask into stripe columns S (true for every row)
    and per-output-block residual columns R_j."""
    D_out, D_in = mask1.shape
    S = np.where(mask1.all(axis=0))[0]
    n_blk = (D_out + OT - 1) // OT
    stripe_set = np.zeros(D_in, dtype=bool)
    stripe_set[S] = True
    R_list = []
    for j in range(n_blk):
        blk = mask1[j * OT:(j + 1) * OT]
        cols = np.where(blk.any(axis=0) & ~stripe_set)[0]
        assert len(cols) <= 128, f"band block {j} has {len(cols)} cols"
        R_list.append(cols)
    return S, R_list


def _build_program(meta):
    nS, R_lens = meta["nS"], meta["R_lens"]
    P_pad = meta["P_pad"]
    Bc = meta["Bc"]
    D1, H, C = meta["D1"], meta["H"], meta["C"]
    n_blk = len(R_lens)
    n_sc = (nS + 127) // 128              # stripe K-chunks
    sc_w = -(-nS // n_sc)                 # stripe chunk width (padded)
    n_kc2 = D1 // OT                      # layer-2 K chunks (= n_blk)
    n_sup = Bc // SC                      # DMA superchunks (= batch chunks)
    use_dr = FP8 and n_sc == 2 and nS == 2 * sc_w

    XD = F8 if FP8 else F32R              # layer-1/2 operand dtype

    nc = bacc.Bacc("TRN2", target_bir_lowering=False, debug=False,
                   enable_asserts=False, num_devices=N_CORES)

    xs_d = nc.dram_tensor("xs", [sc_w, n_sup, n_sc, SC], XD,
                          kind="ExternalInput").ap()
    xb_d = nc.dram_tensor("xb", [P_pad, n_sup, n_blk, SC], XD,
                          kind="ExternalInput").ap()
    ws_d = nc.dram_tensor("ws", [sc_w, n_sc, D1], XD,
                          kind="ExternalInput").ap()
    wb_d = nc.dram_tensor("wb", [P_pad, n_blk * OT], XD,
                          kind="ExternalInput").ap()
    w2_d = nc.dram_tensor("w2", [OT, n_kc2, H], XD, kind="ExternalInput").ap()
    w3_d = nc.dram_tensor("w3", [H, C], F32R, kind="ExternalInput").ap()
    b2_d = nc.dram_tensor("b2", [H, 1], F32, kind="ExternalInput").ap()
    b3_d = nc.dram_tensor("b3", [C, 1], F32, kind="ExternalInput").ap()
    ones_d = nc.dram_tensor("ones", [C, C], F32R, kind="ExternalInput").ap()
    out_d = nc.dram_tensor("out", [C, Bc], F32, kind="ExternalOutput").ap()


    with tile.TileContext(nc) as tc:
        with tc.tile_pool(name="wp", bufs=1) as wp, \
             tc.tile_pool(name="xp", bufs=8) as xp, \
             tc.tile_pool(name="hp", bufs=2) as hp, \
             tc.tile_pool(name="ep", bufs=2) as ep, \
             tc.tile_pool(name="ps1", bufs=5, space="PSUM") as ps1, \
             tc.tile_pool(name="ps2", bufs=1, space="PSUM") as ps2, \
             tc.tile_pool(name="ps3", bufs=1, space="PSUM") as ps3:

            # ---- small constants on the sync ring (fast to start, tiny)
            w3_sb = wp.tile([H, C], F32R)
            nc.sync.dma_start(w3_sb[:], w3_d[:])
            b2_sb = wp.tile([H, 1], F32)
            nc.sync.dma_start(b2_sb[:], b2_d[:])
            b3_sb = wp.tile([C, 1], F32)
            nc.sync.dma_start(b3_sb[:], b3_d[:])
            ones_sb = wp.tile([C, C], F32R)
            nc.sync.dma_start(ones_sb[:], ones_d[:])

            # ---- PE warmup: matmuls on a zeroed scratch tile keep the PE
            # busy through the framework preamble + first DMA transfers, so
            # the HAM clock gate is at 8/8 before real work and real matmuls
            # never run at the cold 1.2 GHz rate. They rotate through the
            # same ps1 bank pool as layer-1 (all complete before real use).
            warm_x = wp.tile([128, NB], XD)
            nc.vector.memset(warm_x[:], 0.5)
            for w in range(N_WARM):
                pw = ps1.tile([OT, NB], F32, tag="l1", name="p1")
                nc.tensor.matmul(pw[:], warm_x[:, :OT], warm_x[:],
                                 start=True, stop=True)

            # ---- resident weights + x superchunks, one SWDGE queue, emitted
            # in consumption order. Each dma_start costs ~750ns of queue
            # issue time, so the transfers coarsen progressively: single
            # chunks (the first split so band block 0 starts as soon as its
            # slice lands — subtile deps give per-region dependencies),
            # then a 2-chunk batch, then 4-chunk batches.
            ws_sb = wp.tile([sc_w, n_sc, D1], XD)
            nc.gpsimd.dma_start(ws_sb[:], ws_d[:])

            xs_tiles, xb_tiles = [], []
            xs_t0 = xp.tile([sc_w, n_sc, SC], XD, name="xs_t0", tag="xs1",
                            bufs=2)
            nc.gpsimd.dma_start(xs_t0[:], xs_d[:, 0])
            xs_tiles.append(xs_t0)

            wb_sb = wp.tile([P_pad, n_blk * OT], XD)
            nc.gpsimd.dma_start(wb_sb[:], wb_d[:])

            xb_t0 = xp.tile([P_pad, n_blk, SC], XD, name="xb_t0", tag="xb1",
                            bufs=2)
            nc.gpsimd.dma_start(xb_t0[:, 0:3], xb_d[:, 0, 0:3])
            xb_tiles.append(xb_t0)

            w2_sb = wp.tile([OT, n_kc2, H], XD)
            nc.gpsimd.dma_start(w2_sb[:], w2_d[:])

            nc.gpsimd.dma_start(xb_t0[:, 3:n_blk], xb_d[:, 0, 3:n_blk])

            xs_t1 = xp.tile([sc_w, n_sc, SC], XD, name="xs_t1", tag="xs1",
                            bufs=2)
            nc.gpsimd.dma_start(xs_t1[:], xs_d[:, 1])
            xs_tiles.append(xs_t1)
            xb_t1 = xp.tile([P_pad, n_blk, SC], XD, name="xb_t1", tag="xb1",
                            bufs=2)
            nc.gpsimd.dma_start(xb_t1[:], xb_d[:, 1])
            xb_tiles.append(xb_t1)

            for s0, s1, tg in ((2, 4, "b2"), (4, 8, "b4a"), (8, 12, "b4b"),
                               (12, 16, "b4c")):
                nb_c = s1 - s0
                xs_tb = xp.tile([sc_w, nb_c, n_sc, SC], XD,
                                name=f"xs_{tg}", tag=f"xs{tg}", bufs=1)
                nc.gpsimd.dma_start(xs_tb[:], xs_d[:, s0:s1])
                xb_tb = xp.tile([P_pad, nb_c, n_blk, SC], XD,
                                name=f"xb_{tg}", tag=f"xb{tg}", bufs=1)
                nc.gpsimd.dma_start(xb_tb[:], xb_d[:, s0:s1])
                for s in range(s0, s1):
                    xs_tiles.append(xs_tb[:, s - s0])
                    xb_tiles.append(xb_tb[:, s - s0])

            # main loop processes chunk PAIRS: layers 1-3 per chunk, then
            # one batched log-softmax epilogue over both chunks' logits
            # ([10, 2, 512] APs spanning two PSUM banks) — halves the
            # per-op overhead + semaphore cost of the small epilogue ops.
            # lse via all-ones matmuls col-tiled into partitions 32-41 of
            # the same PSUM tile (tile_position=(0,32)): one MM sums exp()
            # across the 10 class partitions AND broadcasts the fp32 sum.
            for t in range(n_sup // 2):
                p3t = ps3.tile([48, 2, NB], F32, tag="l3", name="p3t")
                for u in (0, 1):
                    s = 2 * t + u
                    xs_t, xb_t = xs_tiles[s], xb_tiles[s]

                    # ---- layer 1: stripe (one DoubleRow matmul, K=204) +
                    # band (one fp8 matmul, K=R_j+1; the +1 row is an
                    # all-ones x row against the bias, so evictions are
                    # pure ReLU, alternating DVE / ACT).
                    y1_all = hp.tile([OT, n_blk, NB], XD, tag="y1all")
                    for j in range(n_blk):
                        p = ps1.tile([OT, NB], F32, tag="l1", name="p1")
                        if use_dr:
                            nc.tensor.matmul(
                                p[:], ws_sb[:, :, j * OT:(j + 1) * OT],
                                xs_t[:, :, :], start=True, stop=False,
                                perf_mode=DR)
                        else:
                            for c in range(n_sc):
                                kw = nS - c * sc_w if c == n_sc - 1 else sc_w
                                nc.tensor.matmul(
                                    p[:], ws_sb[:kw, c, j * OT:(j + 1) * OT],
                                    xs_t[:kw, c, :],
                                    start=(c == 0), stop=False)
                        nc.tensor.matmul(
                            p[:], wb_sb[:R_lens[j] + 1, j * OT:(j + 1) * OT],
                            xb_t[:R_lens[j] + 1, j, :],
                            start=False, stop=True)
                        if j % 2 == 0:
                            nc.vector.tensor_scalar_max(y1_all[:, j, :],
                                                        p[:], 0.0)
                        else:
                            nc.scalar.activation(y1_all[:, j, :], p[:],
                                                 AF.Relu)

                    # ---- layer 2: DoubleRow pairs of 112-row blocks (K=224)
                    p2 = ps2.tile([H, NB], F32, tag="l2", name="p2")
                    if use_dr:
                        n_pair = n_kc2 // 2
                        for c in range(n_pair):
                            nc.tensor.matmul(
                                p2[:], w2_sb[:, 2 * c:2 * c + 2, :],
                                y1_all[:, 2 * c:2 * c + 2, :],
                                start=(c == 0),
                                stop=(n_kc2 % 2 == 0 and c == n_pair - 1),
                                perf_mode=DR)
                        if n_kc2 % 2:
                            nc.tensor.matmul(p2[:], w2_sb[:, n_kc2 - 1, :],
                                             y1_all[:, n_kc2 - 1, :],
                                             start=False, stop=True)
                    else:
                        for k in range(n_kc2):
                            nc.tensor.matmul(p2[:], w2_sb[:, k, :],
                                             y1_all[:, k, :], start=(k == 0),
                                             stop=(k == n_kc2 - 1))
                    y2 = hp.tile([H, NB], F32R, tag="y2")
                    nc.scalar.activation(y2[:], p2[:], AF.Relu,
                                         bias=b2_sb[:, 0:1])

                    # ---- layer 3 into partitions 0-9 of the shared tile
                    nc.tensor.matmul(p3t[0:C, u, :], w3_sb[:], y2[:],
                                     start=True, stop=True)

                # ---- batched log-softmax over the pair. Bias-add fuses
                # into Exp (ACT) and the final subtract (DVE).
                ex = hp.tile([C, 2, NB], F32R, tag="ex")
                nc.scalar.activation(ex[:], p3t[0:C, :, :], AF.Exp,
                                     bias=b3_sb[:, 0:1])
                for u in (0, 1):
                    nc.tensor.matmul(p3t[32:32 + C, u, :], ones_sb[:],
                                     ex[:, u, :], start=True, stop=True,
                                     tile_position=(0, 32))
                ls = hp.tile([C, 2, NB], F32, tag="ls")
                nc.scalar.activation(ls[:], p3t[32:32 + C, :, :], AF.Ln)
                o = ep.tile([C, 2, NB], F32, tag="o")
                nc.vector.scalar_tensor_tensor(o[:], p3t[0:C, :, :],
                                               b3_sb[:, 0:1], ls[:],
                                               op0=ALU.add,
                                               op1=ALU.subtract)
                for u in (0, 1):
                    bs = (2 * t + u) * SC
                    nc.sync.dma_start(out_d[:, bs:bs + NB], o[:, u, :])

    nc.compile()
    return nc


_CACHE = {}


def _prepare(x, W1, b1, W2, b2, W3, b3, mask1, mask2, mask3):
    B, D1 = x.shape
    H = W2.shape[0]
    C = W3.shape[0]
    assert B % N_CORES == 0
    Bc = B // N_CORES

    S, R_list = _decompose_mask1(np.asarray(mask1))
    R_lens = [len(r) for r in R_list]
    n_blk = len(R_list)
    P_pad = max(R_lens) + 1       # +1: all-ones x row carrying the b1 bias
    nS = len(S)
    n_sc = (nS + 127) // 128
    sc_w = -(-nS // n_sc)

    Wm1 = (np.asarray(W1) * np.asarray(mask1)).astype(np.float32)
    Wm2 = (np.asarray(W2) * np.asarray(mask2)).astype(np.float32)
    Wm3 = (np.asarray(W3) * np.asarray(mask3)).astype(np.float32)

    # stripe weights packed [sc_w, n_sc, D1]
    ws = np.zeros((sc_w, n_sc, D1), np.float32)
    for c in range(n_sc):
        rows = S[c * sc_w:(c + 1) * sc_w]
        ws[:len(rows), c] = Wm1[:, rows].T
    b1f = np.asarray(b1, np.float32)
    wb = np.zeros((P_pad, n_blk * OT), np.float32)
    for j, R in enumerate(R_list):
        wb[:len(R), j * OT:j * OT + OT] = Wm1[j * OT:(j + 1) * OT, R].T
        wb[len(R), j * OT:j * OT + OT] = b1f[j * OT:(j + 1) * OT]
    n_kc2 = D1 // OT
    w2 = np.ascontiguousarray(
        Wm2.T.reshape(n_kc2, OT, H).transpose(1, 0, 2))   # [OT, n_kc2, H]
    w3 = np.ascontiguousarray(Wm3.T)                      # [H, C]
    b2p = np.asarray(b2, np.float32).reshape(H, 1)
    b3p = np.asarray(b3, np.float32).reshape(C, 1)

    xT = np.asarray(x, np.float32).T                      # [D1, B] view
    n_sup = Bc // SC
    # stripe rows packed [sc_w, n_sc, B] then reordered so each per-core
    # superchunk is one contiguous slab: [sc_w, NC, n_sup, n_sc, SC]
    xs_all = np.zeros((sc_w, n_sc, B), np.float32)
    for c in range(n_sc):
        rows = S[c * sc_w:(c + 1) * sc_w]
        xs_all[:len(rows), c] = xT[rows]
    xs_all = np.ascontiguousarray(
        xs_all.reshape(sc_w, n_sc, N_CORES, n_sup, SC)
              .transpose(0, 2, 3, 1, 4))
    xb_all = np.zeros((P_pad, n_blk, B), np.float32)
    for j, R in enumerate(R_list):
        xb_all[:len(R), j] = xT[R]
        xb_all[len(R), j] = 1.0
    xb_all = np.ascontiguousarray(
        xb_all.reshape(P_pad, n_blk, N_CORES, n_sup, SC)
              .transpose(0, 2, 3, 1, 4))

    xw_np = ml_dtypes.float8_e4m3 if FP8 else np.float16
    castx = lambda a: np.asarray(a, dtype=xw_np)
    cast16 = lambda a: np.asarray(a, dtype=np.float16)
    xs_all = castx(xs_all); xb_all = castx(xb_all)
    ws = castx(ws); wb = castx(wb); w2 = castx(w2)
    w3 = cast16(w3)
    meta = dict(nS=nS, R_lens=R_lens, P_pad=P_pad, Bc=Bc, D1=D1, H=H, C=C)
    key = (B, D1, H, C, nS, tuple(R_lens), FP8)
    if key not in _CACHE:
        _CACHE[key] = _build_program(meta)
    nc = _CACHE[key]

    in_maps = []
    for c in range(N_CORES):
        in_maps.append({
            "xs": xs_all[:, c],
            "xb": xb_all[:, c],
            "ws": ws, "wb": wb, "w2": w2, "w3": w3,
            "b2": b2p, "b3": b3p,
            "ones": cast16(np.ones((C, C), np.float32)),
        })
    return nc, in_maps, meta


def _assemble(results, meta):
    outs = [np.ascontiguousarray(results[c]["out"].T)     # [Bc, C]
            for c in range(N_CORES)]
    return np.concatenate(outs, axis=0).astype(np.float32)


def kernel(**inputs):
    nc, in_maps, meta = _prepare(**inputs)
    res = bass_utils.run_bass_kernel_spmd(nc, in_maps,
                                          core_ids=list(range(N_CORES)))
    return _assemble(res.results, meta)


def kernel_traced(tmpdir=None, **inputs):
    """Same as kernel() but with NTFF profiling; returns (output, results)."""
    nc, in_maps, meta = _prepare(**inputs)
    res = bass_utils.run_bass_kernel_spmd(nc, in_maps,
                                          core_ids=list(range(N_CORES)),
                                          trace=True, tmpdir=tmpdir)
    return _assemble(res.results, meta), res
